# revision 43
# baseline (speedup 1.0000x reference)
"""Batched QK^T matmul on 8 Trainium2 NeuronCores.

Problem: mat_0 [8, 2048, 1024] f32, mat_1 [8, 2048, 1024] f32
         out   [8, 2048, 2048] f32 = einsum('bne,bme->bnm')

Sharding: data-parallel over batch — core i computes C = A @ B^T with
A = mat_0[i], B = mat_1[i].

Default mode v3x: host pre-transposes + pre-casts inputs to fp16 in
chunk-major layout [P, e/P, cols]; per core, 64 output tiles [128, 512]
accumulate over 8 contraction chunks in PSUM.  Tiles 16-63 replace the
last 4 (or 2) fp16 chunks with fp8e4 DoubleRow matmuls (2 chunks per
instruction at 2x rate); tiles 0-15 stay fp16 because the kernel head
is input-DMA-bound there and fp8 would buy no wall time.  Global fp8
fraction 0.34375 -> deterministic rel err 1.87e-2 (gate 2e-2).
Measured (8-core): ~114.3us vs 146.9us baseline.

Older modes kept for A/B: v2 (pure fp16), v2m8/12/16 (uniform DR
density), v3 (f=0.3125), fp16t/mix8 (previous generation), fp16x
(on-chip XBAR transpose).
"""

import sys

if "/opt/trn_rl_repo" not in sys.path:
    sys.path.insert(0, "/opt/trn_rl_repo")

import numpy as np

import concourse.mybir as mybir  # noqa: E402
import concourse.tile as tile  # noqa: E402
from concourse import bacc  # noqa: E402
from concourse.bass_utils import run_bass_kernel_spmd  # noqa: E402

P = 128

# Hardcoded problem shape (nn_AttentionMatrix_41841571398230)
B_FULL, N_FULL, M_FULL, E_FULL = 8, 2048, 2048, 1024
FP8_COLS = 256  # contraction cols handled in fp8 for mix8 (one DR pair)


# --------------------------------------------------------------------------
# v2/v3: fp16 + planned-fp8 path, per-round input DMAs, tuned head/tail
# --------------------------------------------------------------------------
# Clock note: the chip intermittently downclocks the PE 2.4 -> ~2.0 GHz
# (512-col matmul stride 216 -> 259 ns) under sustained 8-core benching;
# the state persists for minutes and appears correlated with fp8 density
# and package power/thermals.  Minimizing total PE cycles (fp8 where the
# error budget allows) wins in both clock states.
def qkt_kernel_v2(
    tc, ins, c, n, m, e, dt16, mg=512, fillers=20, dr4=0, dr_plan=None
):
    """C[n, m] (fp16) = A @ B^T, (mostly) fp16.

    ins: a3 [P, e//P, n], b3 [P, e//P, m] fp16 with x3[p, r, col] =
    X^T[r*P + p, col] (host pre-transposed, chunk-major layout so one
    DMA can span several contraction chunks).

    Input DMAs: per-round granularity in consumption order, so the first
    set gates on small early transfers and rounds never outrun arrival.

    dr4: of every 4 output tiles, dr4 handle contraction chunks 6-7 as
    one fp8e4 DoubleRow matmul (ins a8/b8 [P, 2, *]).  Low fp8 density
    avoids the chip power downclock that full mix8 triggers; dr4 sets
    the density/error tradeoff (err ~ 3.2e-2 * sqrt(dr4/16)).
    """
    nc = tc.nc
    f32 = mybir.dt.float32
    f8 = mybir.dt.float8e4
    k16 = e // P
    n_blocks = n // P
    m_groups = m // mg

    with (
        tc.tile_pool(name="ain", bufs=1) as a_pool,
        tc.tile_pool(name="bin", bufs=1) as b_pool,
        tc.tile_pool(name="cst", bufs=1) as cst_pool,
        tc.tile_pool(name="mpsum", bufs=8, space="PSUM") as mpsum_pool,
        tc.tile_pool(name="co", bufs=4) as co_pool,
    ):
        a3, b3 = ins["a3"], ins["b3"]
        ka = a_pool.tile([P, k16, n], dt16, name="ka")
        kb = b_pool.tile([P, k16, m], dt16, name="kb")

        # Input DMAs at per-round granularity in consumption order.  With
        # all 8 cores pulling inputs at once HBM is the constraint, so
        # set 0's per-round needs (kb g0-slice 128KB + ka half 256KB,
        # ~1.06us across 8 cores) must arrive ahead of the 1.73us/round
        # consumption; bulk data follows in tranches sized so later sets'
        # first needs are never behind a multi-MB transfer.
        half_n = n // 2
        for r in range(k16):
            nc.sync.dma_start(kb[:, r : r + 1, 0:mg], b3[:, r : r + 1, 0:mg])
        # A first halves: singles for the first rounds (fine-grain where
        # consumption is imminent), pairs after — 8 singles put too many
        # sems on the scalar queue (~2us serial processing each) and
        # set 1's data lags.
        for r in range(4):
            nc.scalar.dma_start(
                ka[:, r : r + 1, 0:half_n], a3[:, r : r + 1, 0:half_n]
            )
        for r in range(4, k16, 2):
            nc.scalar.dma_start(
                ka[:, r : r + 2, 0:half_n], a3[:, r : r + 2, 0:half_n]
            )
        # set 1 (a second halves, pairs of rounds), then sets 2+ (b
        # remaining col groups)
        for r in range(0, k16, 2):
            nc.scalar.dma_start(
                ka[:, r : r + 2, half_n:n], a3[:, r : r + 2, half_n:n]
            )
        # B bulk (g1, needed only from ~40us): PACED.  HWDGE descriptors
        # drain as soon as the ring is free, and the 16 SDMA engines are
        # shared across rings, so ungated bulk steals HBM bandwidth from
        # the still-critical set-0/1 fine feed (all 8 cores saturate HBM
        # in the first ~20us).  A tiny WAW-gate copy sourced from the A
        # r3 slice (arrives ~19us) holds the bulk triggers until the
        # fine feed is done.
        nc.vector.tensor_copy(kb[:, 0:1, mg : mg + 4], ka[:, 3:4, 0:4])
        nc.vector.tensor_copy(
            kb[:, 4:5, mg : mg + 4], ka[:, 3:4, 4:8]
        )
        nc.sync.dma_start(kb[:, 0:4, mg : 2 * mg], b3[:, 0:4, mg : 2 * mg])
        nc.sync.dma_start(
            kb[:, 4:k16, mg : 2 * mg], b3[:, 4:k16, mg : 2 * mg]
        )
        # g2+g3 (needed from ~55us) ride the SWDGE ring after the fp8
        # tensors, off the busy HWDGE rings entirely.
        nc.gpsimd.dma_start(kb[:, :, 2 * mg : m], b3[:, :, 2 * mg : m])
        if dr4 or dr_plan:
            t8a = a_pool.tile([P, 2, n], f8, name="t8a")
            t8b = b_pool.tile([P, 2, m], f8, name="t8b")
            nc.gpsimd.dma_start(t8b, ins["b8"][:])
            nc.gpsimd.dma_start(t8a, ins["a8"][:])
        if dr_plan and max(dr_plan) > 1:
            t82a = a_pool.tile([P, 2, n], f8, name="t82a")
            t82b = b_pool.tile([P, 2, m], f8, name="t82b")
            nc.gpsimd.dma_start(t82b, ins["b82"][:])
            nc.gpsimd.dma_start(t82a, ins["a82"][:])

        # Warm-up fillers: ramp the PE clock (HAM) and keep it busy until
        # the first input chunks' DMA semaphores mature.
        if fillers:
            ft = cst_pool.tile([P, mg], dt16)
            nc.vector.memset(ft, 0.0)
            fps = mpsum_pool.tile([P, mg], f32, tag="ps", name="fps")
            for _ in range(fillers):
                nc.tensor.matmul(fps, ft[:, :P], ft, start=True, stop=True)

        # Units g-major; 7 sets of 8 PSUM banks, then 4+2+1+1 tail sets
        # so the final drains are small and fit the engines in parallel.
        units = [(g, nb) for g in range(m_groups) for nb in range(n_blocks)]
        sets = [units[i : i + 8] for i in range(0, len(units) - 8, 8)]
        sets += [units[-8:-4], units[-4:-2], units[-2:-1], units[-1:]]
        uidx = 0
        for si, su in enumerate(sets):
            if dr_plan:
                npair = [dr_plan[uidx + u] for u in range(len(su))]
            else:
                npair = [1 if (uidx + u) % 4 < dr4 else 0 for u in range(len(su))]
            uidx += len(su)
            pss = [None] * len(su)
            for r in range(k16):
                for u, (g, nb) in enumerate(su):
                    nf16 = k16 - 2 * npair[u]
                    if r == 0:
                        pss[u] = mpsum_pool.tile(
                            [P, mg], f32, tag="ps", name=f"ps{si}_{u}"
                        )
                    if r < nf16:
                        nc.tensor.matmul(
                            pss[u],
                            ka[:, r, nb * P : (nb + 1) * P],
                            kb[:, r, g * mg : (g + 1) * mg],
                            start=(r == 0),
                            stop=(r == k16 - 1),
                        )
                    elif r == nf16:
                        # first fp8 pair: chunks 4-5 if two pairs, 6-7 if one
                        pa, pb = (t82a, t82b) if npair[u] == 2 else (t8a, t8b)
                        nc.tensor.matmul(
                            pss[u],
                            pa[:, :, nb * P : (nb + 1) * P],
                            pb[:, :, g * mg : (g + 1) * mg],
                            start=False,
                            stop=(npair[u] == 1),
                            perf_mode=mybir.MatmulPerfMode.DoubleRow,
                        )
                    elif r == nf16 + 1 and npair[u] == 2:
                        nc.tensor.matmul(
                            pss[u],
                            t8a[:, :, nb * P : (nb + 1) * P],
                            t8b[:, :, g * mg : (g + 1) * mg],
                            start=False,
                            stop=True,
                            perf_mode=mybir.MatmulPerfMode.DoubleRow,
                        )
            if len(su) <= 2:
                # Tail sets: each unit drains as column halves on both
                # engines in parallel, each half DMA'd on its own queue,
                # so the critical chain after the very last matmul is a
                # half-width copy + one small DMA.
                hw = mg // 2
                for j, (g, nb) in enumerate(su):
                    ot = co_pool.tile([P, mg], dt16, tag="ott")
                    nc.vector.tensor_copy(ot[:, 0:hw], pss[j][:, 0:hw])
                    nc.scalar.copy(ot[:, hw:mg], pss[j][:, hw:mg])
                    nc.sync.dma_start(
                        c[:, nb : nb + 1, g * mg : g * mg + hw], ot[:, 0:hw]
                    )
                    nc.scalar.dma_start(
                        c[:, nb : nb + 1, g * mg + hw : (g + 1) * mg],
                        ot[:, hw:mg],
                    )
            else:
                for half, qh in ((0, nc.sync), (4, nc.scalar)):
                    hu = su[half : half + 4]
                    if not hu:
                        continue
                    ot = co_pool.tile([P, 4, mg], dt16, tag="ot")
                    for j, (g, nb) in enumerate(hu):
                        if j % 2 == 0:
                            nc.vector.tensor_copy(ot[:, j, :], pss[half + j])
                        else:
                            nc.scalar.copy(ot[:, j, :], pss[half + j])
                    gh, nbh = hu[0]
                    qh.dma_start(
                        c[:, nbh : nbh + 4, gh * mg : (gh + 1) * mg], ot
                    )


# --------------------------------------------------------------------------
# Pre-transposed path (fp16t / mix8): inputs land in matmul-ready layout
# --------------------------------------------------------------------------
def qkt_kernel_pret(tc, ins, c, n, m, e, dt16, fp8, mg=512, fillers=None):
    """C[n, m] (fp16) = A @ B^T given host-pretransposed operands.

    ins: a16 [e16, n], b16 [e16, m] fp16; if fp8 also a8, b8 shaped
    [P, 2, n] / [P, 2, m] fp8e4 holding contraction cols 0:256 in
    DoubleRow-interleaved layout ([p, i, r] = X[r, i*P + p]).

    Structure: 64 output tiles [P, mg] in 8 sets of 8 PSUM banks.
    Per set, k-outer emission (one round per contraction chunk across
    all 8 banks) so matmuls start as soon as the first chunks land.
    DMA issue order tracks consumption order; chunk loads are split in
    column halves so the first set is not gated on whole-tensor loads.
    """
    nc = tc.nc
    f32 = mybir.dt.float32
    f8 = mybir.dt.float8e4
    e16 = e - (FP8_COLS if fp8 else 0)
    k16 = e16 // P  # fp16 contraction chunks (8 or 6)
    n_blocks = n // P
    m_groups = m // mg
    rounds = k16 + (1 if fp8 else 0)
    if fillers is None:
        fillers = 18

    with (
        tc.tile_pool(name="ain", bufs=1) as a_pool,
        tc.tile_pool(name="bin", bufs=1) as b_pool,
        tc.tile_pool(name="cst", bufs=1) as cst_pool,
        tc.tile_pool(name="mpsum", bufs=8, space="PSUM") as mpsum_pool,
        tc.tile_pool(name="co", bufs=4) as co_pool,
    ):
        a16, b16 = ins["a16"], ins["b16"]

        # Warm-up fillers: ramp the PE clock and cover the first chunk's
        # DMA-semaphore latency (~6us after the transfer itself).
        # Content is a memset tile; never read.
        if fillers:
            ft = cst_pool.tile([P, mg], dt16)
            nc.gpsimd.memset(ft, 0.0)
            fps = mpsum_pool.tile([P, mg], f32, tag="ps", name="fps")
            for _ in range(fillers):
                nc.tensor.matmul(fps, ft[:, :P], ft, start=True, stop=True)

        # Chunk loads in consumption order (round r consumes pair r; the
        # fp8 pair feeds the LAST round of every set so it can load
        # late).  Completion semaphores process serially per queue
        # (~1.5us each, ~6us pipeline latency), so the two fast HWDGE
        # queues each carry one tensor's chunks — sem cadence 1.5us per
        # round vs the PE's 1.73us round time — and the slow SWDGE
        # gpsimd queue carries only the late-needed fp8 pair.
        ka = [a_pool.tile([P, n], dt16, name=f"ka{k}") for k in range(k16)]
        kb = [b_pool.tile([P, m], dt16, name=f"kb{k}") for k in range(k16)]
        if fp8:
            t8a = a_pool.tile([P, 2, n], f8, name="t8a")
            t8b = b_pool.tile([P, 2, m], f8, name="t8b")
        # k=0 is split so the exact sub-regions round 0 of set 0 reads
        # (kb0 g=0 slice, ka0 nb 0..7 half) land — and their completion
        # semaphores mature — as early as possible.
        nc.sync.dma_start(kb[0][:, :mg], b16[0:P, :mg])
        nc.scalar.dma_start(ka[0][:, : n // 2], a16[0:P, : n // 2])
        nc.sync.dma_start(kb[0][:, mg:], b16[0:P, mg:])
        nc.scalar.dma_start(ka[0][:, n // 2 :], a16[0:P, n // 2 :])
        for k in range(1, k16):
            nc.sync.dma_start(kb[k], b16[k * P : (k + 1) * P, :])
            nc.scalar.dma_start(ka[k], a16[k * P : (k + 1) * P, :])
        if fp8:
            nc.gpsimd.dma_start(t8b, ins["b8"][:])
            nc.gpsimd.dma_start(t8a, ins["a8"][:])

        # Unit order: g-major, nb within; sets of 8 units = 8 PSUM banks.
        # The final 8 units run as two 4-unit sets so the very last
        # drain (on the critical tail) covers fewer copies.
        units = [(g, nb) for g in range(m_groups) for nb in range(n_blocks)]
        sets = [units[i : i + 8] for i in range(0, len(units) - 8, 8)]
        sets += [units[-8:-4], units[-4:]]
        for si, su in enumerate(sets):
            # PSUM tiles allocated lazily (at first use) so each round-0
            # matmul waits only on its own bank's drain, not all eight.
            pss = [None] * len(su)
            for r in range(rounds):
                if fp8 and r == rounds - 1:
                    for u, (g, nb) in enumerate(su):
                        nc.tensor.matmul(
                            pss[u],
                            t8a[:, :, nb * P : (nb + 1) * P],
                            t8b[:, :, g * mg : (g + 1) * mg],
                            start=False,
                            stop=True,
                            perf_mode=mybir.MatmulPerfMode.DoubleRow,
                        )
                else:
                    for u, (g, nb) in enumerate(su):
                        if r == 0:
                            pss[u] = mpsum_pool.tile(
                                [P, mg], f32, tag="ps", name=f"ps{si}_{u}"
                            )
                        nc.tensor.matmul(
                            pss[u],
                            ka[r][:, nb * P : (nb + 1) * P],
                            kb[r][:, g * mg : (g + 1) * mg],
                            start=(r == 0),
                            stop=(r == rounds - 1),
                        )
            # Drain: copies alternate vector/scalar into merged staging
            # tiles; one output DMA per 4 units (few large DMAs keep slow
            # DMA-completion semaphores off the critical path).  The last
            # set instead streams per-unit DMAs so the final transfer is
            # small and starts as early as possible (shortens the tail).
            # c is laid out [P, n_blocks, m]: c[p, nb, col] = C[nb*P+p, col]
            for half, qh in ((0, nc.sync), (4, nc.scalar)):
                hu = su[half : half + 4]
                if not hu:
                    continue
                if len(su) == 4 and si == len(sets) - 1:
                    # Final 4-unit set: issue on scalar so the last two
                    # output DMAs ride different queues and overlap.
                    qh = nc.scalar
                ot = co_pool.tile([P, 4, mg], dt16, tag="ot")
                for j, (g, nb) in enumerate(hu):
                    if j % 2 == 0:
                        nc.vector.tensor_copy(ot[:, j, :], pss[half + j])
                    else:
                        nc.scalar.copy(ot[:, j, :], pss[half + j])
                gh, nbh = hu[0]
                qh.dma_start(
                    c[:, nbh : nbh + 4, gh * mg : (gh + 1) * mg], ot
                )


# --------------------------------------------------------------------------
# XBAR path (fp16x): host pre-cast fp16, on-chip DMA-transpose (fallback)
# --------------------------------------------------------------------------
def qkt_kernel_xbar(tc, ins, c, n, m, e, dt16, mg=512):
    nc = tc.nc
    f32 = mybir.dt.float32
    mg = min(mg, m)
    n_blocks = n // P
    e_chunks = e // P
    m_groups = m // mg
    a_hi, b_hi = ins
    srcs = {"ah": a_hi, "bh": b_hi}

    with (
        tc.tile_pool(name="tpt", bufs=1) as tp_pool,
        tc.tile_pool(name="mpsum", bufs=8, space="PSUM") as mpsum_pool,
        tc.tile_pool(name="co", bufs=4) as co_pool,
    ):
        tchunks = {tag: [None] * e_chunks for tag in srcs}
        for k in range(e_chunks):
            for tag, src in srcs.items():
                rows = n if tag[0] == "a" else m
                t = tp_pool.tile([P, rows], dt16, name=f"t_{tag}{k}")
                nc.sync.dma_start_transpose(t, src[:, k * P : (k + 1) * P])
                tchunks[tag][k] = t

        units = [(g, nb) for g in range(m_groups) for nb in range(n_blocks)]
        sets = [units[i0 : i0 + 8] for i0 in range(0, len(units), 8)]
        for si, chunk_units in enumerate(sets):
            pss = [
                mpsum_pool.tile([P, mg], f32, tag="ps", name=f"ps_{si}_{u}")
                for u in range(len(chunk_units))
            ]
            for k in range(e_chunks):
                for u, (g, nb) in enumerate(chunk_units):
                    nc.tensor.matmul(
                        pss[u],
                        tchunks["ah"][k][:, nb * P : (nb + 1) * P],
                        tchunks["bh"][k][:, g * mg : (g + 1) * mg],
                        start=(k == 0),
                        stop=(k == e_chunks - 1),
                    )
            for u, (g, nb) in enumerate(chunk_units):
                ot = co_pool.tile([P, mg], f32, tag="ot")
                nc.vector.tensor_copy(ot, pss[u])
                nc.scalar.dma_start(
                    c[nb * P : (nb + 1) * P, g * mg : (g + 1) * mg], ot
                )


# --------------------------------------------------------------------------
# Builders
# --------------------------------------------------------------------------
def build_qkt(n, m, e, mm_mode="fp16t", mg=512, fillers=None):
    f32 = mybir.dt.float32
    f16 = mybir.dt.float16
    f8 = mybir.dt.float8e4
    nc = bacc.Bacc(None, target_bir_lowering=False)
    with tile.TileContext(nc) as tc:
        with tc.tile_pool(name="dram", bufs=1, space="DRAM") as dram:
            if mm_mode in ("v2", "v2m8", "v2m12", "v2m16", "v3", "v3x",
                           "v3z"):
                dr4 = {"v2": 0, "v2m8": 2, "v2m12": 3, "v2m16": 4,
                       "v3": 0, "v3x": 0, "v3z": 0}[mm_mode]
                # v3: sets 0-1 (tiles 0-15) stay fp16 (they are DMA-feed
                # bound at kernel start — fp8 there saves no wall time);
                # tiles 16+ get 2 fp8 pairs, every 3rd gets 1, spending
                # the error budget where the stream is compute-bound.
                # Global fp8 fraction 80*256/65536 = 0.3125 -> rel err
                # ~1.78e-2 (deterministic), gate 2e-2.
                dr_plan = None
                if mm_mode == "v3":
                    dr_plan = [
                        0 if i < 16 else (1 if (i - 16) % 3 == 2 else 2)
                        for i in range(64)
                    ]
                elif mm_mode == "v3z":
                    # all 48 late tiles with 2 pairs: f=0.375, rel err
                    # 1.594e-2*sqrt(0.375/0.25) = 1.951e-2 (< 2e-2,
                    # deterministic)
                    dr_plan = [0 if i < 16 else 2 for i in range(64)]
                elif mm_mode == "v3x":
                    # 40 late tiles with 2 pairs, 8 with 1: f=0.34375,
                    # rel err 1.594e-2*sqrt(f/0.25) = 1.87e-2 (< 2e-2,
                    # deterministic)
                    dr_plan = [
                        0 if i < 16 else (1 if (i - 16) % 6 == 5 else 2)
                        for i in range(64)
                    ]
                c = dram.tile(
                    [P, n // P, m], f16, kind="ExternalOutput", name="out"
                )
                handles = {
                    "a3": dram.tile(
                        [P, e // P, n], f16, kind="ExternalInput", name="a3"
                    ),
                    "b3": dram.tile(
                        [P, e // P, m], f16, kind="ExternalInput", name="b3"
                    ),
                }
                if dr4 or dr_plan:
                    handles["a8"] = dram.tile(
                        [P, 2, n], f8, kind="ExternalInput", name="a8"
                    )
                    handles["b8"] = dram.tile(
                        [P, 2, m], f8, kind="ExternalInput", name="b8"
                    )
                if dr_plan and max(dr_plan) > 1:
                    handles["a82"] = dram.tile(
                        [P, 2, n], f8, kind="ExternalInput", name="a82"
                    )
                    handles["b82"] = dram.tile(
                        [P, 2, m], f8, kind="ExternalInput", name="b82"
                    )
                qkt_kernel_v2(
                    tc,
                    {k: h[:] for k, h in handles.items()},
                    c[:],
                    n,
                    m,
                    e,
                    f16,
                    mg=mg,
                    fillers=fillers if fillers is not None else 20,
                    dr4=dr4,
                    dr_plan=dr_plan,
                )
                in_names = {k: h.name for k, h in handles.items()}
            elif mm_mode in ("fp16t", "mix8"):
                fp8 = mm_mode == "mix8"
                e16 = e - (FP8_COLS if fp8 else 0)
                c = dram.tile(
                    [P, n // P, m], f16, kind="ExternalOutput", name="out"
                )
                handles = {
                    "a16": dram.tile([e16, n], f16, kind="ExternalInput", name="a16"),
                    "b16": dram.tile([e16, m], f16, kind="ExternalInput", name="b16"),
                }
                if fp8:
                    handles["a8"] = dram.tile(
                        [P, 2, n], f8, kind="ExternalInput", name="a8"
                    )
                    handles["b8"] = dram.tile(
                        [P, 2, m], f8, kind="ExternalInput", name="b8"
                    )
                if dr_plan and max(dr_plan) > 1:
                    handles["a82"] = dram.tile(
                        [P, 2, n], f8, kind="ExternalInput", name="a82"
                    )
                    handles["b82"] = dram.tile(
                        [P, 2, m], f8, kind="ExternalInput", name="b82"
                    )
                qkt_kernel_pret(
                    tc,
                    {k: h[:] for k, h in handles.items()},
                    c[:],
                    n,
                    m,
                    e,
                    f16,
                    fp8,
                    mg=mg,
                    fillers=fillers,
                )
                in_names = {k: h.name for k, h in handles.items()}
            elif mm_mode == "fp16x":
                c = dram.tile([n, m], f32, kind="ExternalOutput", name="out")
                a = dram.tile([n, e], f16, kind="ExternalInput", name="a_hi")
                b = dram.tile([m, e], f16, kind="ExternalInput", name="b_hi")
                qkt_kernel_xbar(tc, [a[:], b[:]], c[:], n, m, e, f16, mg=mg)
                in_names = [a.name, b.name]
            else:
                raise ValueError(f"unknown mode {mm_mode}")
    nc.compile()
    return nc, in_names, c.name


_CACHE = {}


def _get_built(n, m, e, mm_mode, mg=512, fillers=None):
    key = (n, m, e, mm_mode, mg, fillers)
    if key not in _CACHE:
        _CACHE[key] = build_qkt(n, m, e, mm_mode=mm_mode, mg=mg, fillers=fillers)
    return _CACHE[key]


def _dr_interleave(xT8):
    """[256, r] fp8 (transposed cols 0:256) -> [128, 2, r] DR layout."""
    return np.ascontiguousarray(np.stack([xT8[:P], xT8[P : 2 * P]], axis=1))


def _pair_chunks(xT):
    """[e16, r] -> [e16//256, 128, 2, r]: chunk pairs, partition-major."""
    e16, r = xT.shape
    return np.ascontiguousarray(
        xT.reshape(e16 // (2 * P), 2, P, r).transpose(0, 2, 1, 3)
    )


def prep_inputs(mat_0, mat_1, mm_mode, in_names):
    """Host-side per-core input prep for each mode."""
    import ml_dtypes

    bsz = mat_0.shape[0]
    f16 = np.float16
    if mm_mode == "fp16x":
        a16 = mat_0.astype(f16)
        b16 = mat_1.astype(f16)
        return [{in_names[0]: a16[i], in_names[1]: b16[i]} for i in range(bsz)]

    f8 = ml_dtypes.float8_e4m3
    maps = []
    for i in range(bsz):
        aT = mat_0[i].T  # [e, n]
        bT = mat_1[i].T  # [e, m]
        if mm_mode in ("v2", "v2m8", "v2m12", "v2m16", "v3", "v3x", "v3z"):
            # [e, r] -> [P, e//P, r]: x3[p, k, col] = xT[k*P + p, col]
            n_, m_ = aT.shape[1], bT.shape[1]
            a3 = np.ascontiguousarray(
                aT.astype(f16).reshape(-1, P, n_).transpose(1, 0, 2)
            )
            b3 = np.ascontiguousarray(
                bT.astype(f16).reshape(-1, P, m_).transpose(1, 0, 2)
            )
            mp = {in_names["a3"]: a3, in_names["b3"]: b3}
            if mm_mode in ("v2m8", "v2m12", "v2m16", "v3", "v3x", "v3z"):
                e_ = aT.shape[0]
                mp[in_names["a8"]] = _dr_interleave(
                    aT[e_ - 2 * P :].astype(f8)
                )
                mp[in_names["b8"]] = _dr_interleave(
                    bT[e_ - 2 * P :].astype(f8)
                )
            if mm_mode in ("v3", "v3x", "v3z"):
                mp[in_names["a82"]] = _dr_interleave(
                    aT[e_ - 4 * P : e_ - 2 * P].astype(f8)
                )
                mp[in_names["b82"]] = _dr_interleave(
                    bT[e_ - 4 * P : e_ - 2 * P].astype(f8)
                )
            maps.append(mp)
        elif mm_mode == "fp16t":
            maps.append(
                {
                    in_names["a16"]: np.ascontiguousarray(aT.astype(f16)),
                    in_names["b16"]: np.ascontiguousarray(bT.astype(f16)),
                }
            )
        else:  # mix8
            maps.append(
                {
                    in_names["a16"]: np.ascontiguousarray(aT[FP8_COLS:].astype(f16)),
                    in_names["b16"]: np.ascontiguousarray(bT[FP8_COLS:].astype(f16)),
                    in_names["a8"]: _dr_interleave(aT[:FP8_COLS].astype(f8)),
                    in_names["b8"]: _dr_interleave(bT[:FP8_COLS].astype(f8)),
                }
            )
    return maps


def run_qkt(mat_0, mat_1, mm_mode="mix8", mg=512, fillers=None, trace=False):
    """Run the sharded kernel on full inputs [b, n, e], [b, m, e]."""
    bsz, n, e = mat_0.shape
    _, m, _ = mat_1.shape
    nc, in_names, c_name = _get_built(n, m, e, mm_mode, mg, fillers)
    in_maps = prep_inputs(mat_0, mat_1, mm_mode, in_names)
    res = run_bass_kernel_spmd(nc, in_maps, core_ids=list(range(bsz)), trace=trace)

    def unshard(r):
        o = r[c_name]
        if mm_mode in ("fp16t", "mix8", "v2", "v2m8", "v2m12", "v2m16",
                       "v3", "v3x", "v3z"):
            # [P, n_blocks, m] -> [n, m]
            o = o.transpose(1, 0, 2).reshape(n, m)
        return o.astype(np.float32)

    out = np.stack([unshard(res.results[i]) for i in range(bsz)], axis=0)
    return out, res


DEFAULT_MODE = "v3x"


def kernel(mat_0, mat_1):
    out, _ = run_qkt(
        np.asarray(mat_0, dtype=np.float32),
        np.asarray(mat_1, dtype=np.float32),
        mm_mode=DEFAULT_MODE,
    )
    return out



# revision 44
# speedup vs baseline: 1.0363x; 1.0363x over previous
"""Batched QK^T matmul on 8 Trainium2 NeuronCores.

Problem: mat_0 [8, 2048, 1024] f32, mat_1 [8, 2048, 1024] f32
         out   [8, 2048, 2048] f32 = einsum('bne,bme->bnm')

Sharding: data-parallel over batch — core i computes C = A @ B^T with
A = mat_0[i], B = mat_1[i].

Default mode v3x: host pre-transposes + pre-casts inputs to fp16 in
chunk-major layout [P, e/P, cols]; per core, 64 output tiles [128, 512]
accumulate over 8 contraction chunks in PSUM.  Tiles 16-63 replace the
last 4 (or 2) fp16 chunks with fp8e4 DoubleRow matmuls (2 chunks per
instruction at 2x rate); tiles 0-15 stay fp16 because the kernel head
is input-DMA-bound there and fp8 would buy no wall time.  Global fp8
fraction 0.34375 -> deterministic rel err 1.87e-2 (gate 2e-2).
Measured (8-core): ~114.3us vs 146.9us baseline.

Older modes kept for A/B: v2 (pure fp16), v2m8/12/16 (uniform DR
density), v3 (f=0.3125), fp16t/mix8 (previous generation), fp16x
(on-chip XBAR transpose).
"""

import sys

if "/opt/trn_rl_repo" not in sys.path:
    sys.path.insert(0, "/opt/trn_rl_repo")

import numpy as np

import concourse.mybir as mybir  # noqa: E402
import concourse.tile as tile  # noqa: E402
from concourse import bacc  # noqa: E402
from concourse.bass_utils import run_bass_kernel_spmd  # noqa: E402

P = 128

# Hardcoded problem shape (nn_AttentionMatrix_41841571398230)
B_FULL, N_FULL, M_FULL, E_FULL = 8, 2048, 2048, 1024
FP8_COLS = 256  # contraction cols handled in fp8 for mix8 (one DR pair)


# --------------------------------------------------------------------------
# v2/v3: fp16 + planned-fp8 path, per-round input DMAs, tuned head/tail
# --------------------------------------------------------------------------
# Clock note: the chip intermittently downclocks the PE 2.4 -> ~2.0 GHz
# (512-col matmul stride 216 -> 259 ns) under sustained 8-core benching;
# the state persists for minutes and appears correlated with fp8 density
# and package power/thermals.  Minimizing total PE cycles (fp8 where the
# error budget allows) wins in both clock states.
def qkt_kernel_v2(
    tc, ins, c, n, m, e, dt16, mg=512, fillers=20, dr4=0, dr_plan=None
):
    """C[n, m] (fp16) = A @ B^T, (mostly) fp16.

    ins: a3 [P, e//P, n], b3 [P, e//P, m] fp16 with x3[p, r, col] =
    X^T[r*P + p, col] (host pre-transposed, chunk-major layout so one
    DMA can span several contraction chunks).

    Input DMAs: per-round granularity in consumption order, so the first
    set gates on small early transfers and rounds never outrun arrival.

    dr4: of every 4 output tiles, dr4 handle contraction chunks 6-7 as
    one fp8e4 DoubleRow matmul (ins a8/b8 [P, 2, *]).  Low fp8 density
    avoids the chip power downclock that full mix8 triggers; dr4 sets
    the density/error tradeoff (err ~ 3.2e-2 * sqrt(dr4/16)).
    """
    nc = tc.nc
    f32 = mybir.dt.float32
    f8 = mybir.dt.float8e4
    k16 = e // P
    n_blocks = n // P
    m_groups = m // mg

    with (
        tc.tile_pool(name="ain", bufs=1) as a_pool,
        tc.tile_pool(name="bin", bufs=1) as b_pool,
        tc.tile_pool(name="cst", bufs=1) as cst_pool,
        tc.tile_pool(name="mpsum", bufs=8, space="PSUM") as mpsum_pool,
        tc.tile_pool(name="co", bufs=4) as co_pool,
    ):
        a3, b3 = ins["a3"], ins["b3"]
        ka = a_pool.tile([P, k16, n], dt16, name="ka")
        kb = b_pool.tile([P, k16, m], dt16, name="kb")

        # Input DMAs at per-round granularity in consumption order.  With
        # all 8 cores pulling inputs at once HBM is the constraint, so
        # set 0's per-round needs (kb g0-slice 128KB + ka half 256KB,
        # ~1.06us across 8 cores) must arrive ahead of the 1.73us/round
        # consumption; bulk data follows in tranches sized so later sets'
        # first needs are never behind a multi-MB transfer.
        half_n = n // 2
        for r in range(k16):
            nc.sync.dma_start(kb[:, r : r + 1, 0:mg], b3[:, r : r + 1, 0:mg])
        # A first halves: singles for the first rounds (fine-grain where
        # consumption is imminent), pairs after — 8 singles put too many
        # sems on the scalar queue (~2us serial processing each) and
        # set 1's data lags.
        for r in range(4):
            nc.scalar.dma_start(
                ka[:, r : r + 1, 0:half_n], a3[:, r : r + 1, 0:half_n]
            )
        for r in range(4, k16, 2):
            nc.scalar.dma_start(
                ka[:, r : r + 2, 0:half_n], a3[:, r : r + 2, 0:half_n]
            )
        # set 1 (a second halves, pairs of rounds), then sets 2+ (b
        # remaining col groups)
        for r in range(0, k16, 2):
            nc.scalar.dma_start(
                ka[:, r : r + 2, half_n:n], a3[:, r : r + 2, half_n:n]
            )
        # B bulk (g1, needed only from ~40us): PACED.  HWDGE descriptors
        # drain as soon as the ring is free, and the 16 SDMA engines are
        # shared across rings, so ungated bulk steals HBM bandwidth from
        # the still-critical set-0/1 fine feed (all 8 cores saturate HBM
        # in the first ~20us).  A tiny WAW-gate copy sourced from the A
        # r3 slice (arrives ~19us) holds the bulk triggers until the
        # fine feed is done.
        nc.vector.tensor_copy(kb[:, 0:1, mg : mg + 4], ka[:, 3:4, 0:4])
        nc.vector.tensor_copy(
            kb[:, 4:5, mg : mg + 4], ka[:, 3:4, 4:8]
        )
        nc.sync.dma_start(kb[:, 0:4, mg : 2 * mg], b3[:, 0:4, mg : 2 * mg])
        nc.sync.dma_start(
            kb[:, 4:k16, mg : 2 * mg], b3[:, 4:k16, mg : 2 * mg]
        )
        # g2+g3 (needed from ~55us) follow on the same ring — the gated
        # tranches ahead of them hold the FIFO, pacing these too.
        nc.sync.dma_start(kb[:, :, 2 * mg : m], b3[:, :, 2 * mg : m])
        if dr4 or dr_plan:
            t8a = a_pool.tile([P, 2, n], f8, name="t8a")
            t8b = b_pool.tile([P, 2, m], f8, name="t8b")
            nc.gpsimd.dma_start(t8b, ins["b8"][:])
            nc.gpsimd.dma_start(t8a, ins["a8"][:])
        if dr_plan and max(dr_plan) > 1:
            t82a = a_pool.tile([P, 2, n], f8, name="t82a")
            t82b = b_pool.tile([P, 2, m], f8, name="t82b")
            nc.gpsimd.dma_start(t82b, ins["b82"][:])
            nc.gpsimd.dma_start(t82a, ins["a82"][:])

        # Warm-up fillers: ramp the PE clock (HAM) and keep it busy until
        # the first input chunks' DMA semaphores mature.
        if fillers:
            ft = cst_pool.tile([P, mg], dt16)
            nc.vector.memset(ft, 0.0)
            fps = mpsum_pool.tile([P, mg], f32, tag="ps", name="fps")
            for _ in range(fillers):
                nc.tensor.matmul(fps, ft[:, :P], ft, start=True, stop=True)

        # Units g-major; 7 sets of 8 PSUM banks, then 4+2+1+1 tail sets
        # so the final drains are small and fit the engines in parallel.
        units = [(g, nb) for g in range(m_groups) for nb in range(n_blocks)]
        sets = [units[i : i + 8] for i in range(0, len(units) - 8, 8)]
        sets += [units[-8:-4], units[-4:-2], units[-2:-1], units[-1:]]
        uidx = 0
        for si, su in enumerate(sets):
            if dr_plan:
                npair = [dr_plan[uidx + u] for u in range(len(su))]
            else:
                npair = [1 if (uidx + u) % 4 < dr4 else 0 for u in range(len(su))]
            uidx += len(su)
            pss = [None] * len(su)
            for r in range(k16):
                for u, (g, nb) in enumerate(su):
                    nf16 = k16 - 2 * npair[u]
                    if r == 0:
                        pss[u] = mpsum_pool.tile(
                            [P, mg], f32, tag="ps", name=f"ps{si}_{u}"
                        )
                    if r < nf16:
                        nc.tensor.matmul(
                            pss[u],
                            ka[:, r, nb * P : (nb + 1) * P],
                            kb[:, r, g * mg : (g + 1) * mg],
                            start=(r == 0),
                            stop=(r == k16 - 1),
                        )
                    elif r == nf16:
                        # first fp8 pair: chunks 4-5 if two pairs, 6-7 if one
                        pa, pb = (t82a, t82b) if npair[u] == 2 else (t8a, t8b)
                        nc.tensor.matmul(
                            pss[u],
                            pa[:, :, nb * P : (nb + 1) * P],
                            pb[:, :, g * mg : (g + 1) * mg],
                            start=False,
                            stop=(npair[u] == 1),
                            perf_mode=mybir.MatmulPerfMode.DoubleRow,
                        )
                    elif r == nf16 + 1 and npair[u] == 2:
                        nc.tensor.matmul(
                            pss[u],
                            t8a[:, :, nb * P : (nb + 1) * P],
                            t8b[:, :, g * mg : (g + 1) * mg],
                            start=False,
                            stop=True,
                            perf_mode=mybir.MatmulPerfMode.DoubleRow,
                        )
            if len(su) <= 2:
                # Tail sets: each unit drains as column halves on both
                # engines in parallel, each half DMA'd on its own queue,
                # so the critical chain after the very last matmul is a
                # half-width copy + one small DMA.
                hw = mg // 2
                for j, (g, nb) in enumerate(su):
                    ot = co_pool.tile([P, mg], dt16, tag="ott")
                    nc.vector.tensor_copy(ot[:, 0:hw], pss[j][:, 0:hw])
                    nc.scalar.copy(ot[:, hw:mg], pss[j][:, hw:mg])
                    nc.sync.dma_start(
                        c[:, nb : nb + 1, g * mg : g * mg + hw], ot[:, 0:hw]
                    )
                    nc.scalar.dma_start(
                        c[:, nb : nb + 1, g * mg + hw : (g + 1) * mg],
                        ot[:, hw:mg],
                    )
            else:
                for half, qh in ((0, nc.sync), (4, nc.scalar)):
                    hu = su[half : half + 4]
                    if not hu:
                        continue
                    ot = co_pool.tile([P, 4, mg], dt16, tag="ot")
                    for j, (g, nb) in enumerate(hu):
                        if j % 2 == 0:
                            nc.vector.tensor_copy(ot[:, j, :], pss[half + j])
                        else:
                            nc.scalar.copy(ot[:, j, :], pss[half + j])
                    gh, nbh = hu[0]
                    qh.dma_start(
                        c[:, nbh : nbh + 4, gh * mg : (gh + 1) * mg], ot
                    )


# --------------------------------------------------------------------------
# Pre-transposed path (fp16t / mix8): inputs land in matmul-ready layout
# --------------------------------------------------------------------------
def qkt_kernel_pret(tc, ins, c, n, m, e, dt16, fp8, mg=512, fillers=None):
    """C[n, m] (fp16) = A @ B^T given host-pretransposed operands.

    ins: a16 [e16, n], b16 [e16, m] fp16; if fp8 also a8, b8 shaped
    [P, 2, n] / [P, 2, m] fp8e4 holding contraction cols 0:256 in
    DoubleRow-interleaved layout ([p, i, r] = X[r, i*P + p]).

    Structure: 64 output tiles [P, mg] in 8 sets of 8 PSUM banks.
    Per set, k-outer emission (one round per contraction chunk across
    all 8 banks) so matmuls start as soon as the first chunks land.
    DMA issue order tracks consumption order; chunk loads are split in
    column halves so the first set is not gated on whole-tensor loads.
    """
    nc = tc.nc
    f32 = mybir.dt.float32
    f8 = mybir.dt.float8e4
    e16 = e - (FP8_COLS if fp8 else 0)
    k16 = e16 // P  # fp16 contraction chunks (8 or 6)
    n_blocks = n // P
    m_groups = m // mg
    rounds = k16 + (1 if fp8 else 0)
    if fillers is None:
        fillers = 18

    with (
        tc.tile_pool(name="ain", bufs=1) as a_pool,
        tc.tile_pool(name="bin", bufs=1) as b_pool,
        tc.tile_pool(name="cst", bufs=1) as cst_pool,
        tc.tile_pool(name="mpsum", bufs=8, space="PSUM") as mpsum_pool,
        tc.tile_pool(name="co", bufs=4) as co_pool,
    ):
        a16, b16 = ins["a16"], ins["b16"]

        # Warm-up fillers: ramp the PE clock and cover the first chunk's
        # DMA-semaphore latency (~6us after the transfer itself).
        # Content is a memset tile; never read.
        if fillers:
            ft = cst_pool.tile([P, mg], dt16)
            nc.gpsimd.memset(ft, 0.0)
            fps = mpsum_pool.tile([P, mg], f32, tag="ps", name="fps")
            for _ in range(fillers):
                nc.tensor.matmul(fps, ft[:, :P], ft, start=True, stop=True)

        # Chunk loads in consumption order (round r consumes pair r; the
        # fp8 pair feeds the LAST round of every set so it can load
        # late).  Completion semaphores process serially per queue
        # (~1.5us each, ~6us pipeline latency), so the two fast HWDGE
        # queues each carry one tensor's chunks — sem cadence 1.5us per
        # round vs the PE's 1.73us round time — and the slow SWDGE
        # gpsimd queue carries only the late-needed fp8 pair.
        ka = [a_pool.tile([P, n], dt16, name=f"ka{k}") for k in range(k16)]
        kb = [b_pool.tile([P, m], dt16, name=f"kb{k}") for k in range(k16)]
        if fp8:
            t8a = a_pool.tile([P, 2, n], f8, name="t8a")
            t8b = b_pool.tile([P, 2, m], f8, name="t8b")
        # k=0 is split so the exact sub-regions round 0 of set 0 reads
        # (kb0 g=0 slice, ka0 nb 0..7 half) land — and their completion
        # semaphores mature — as early as possible.
        nc.sync.dma_start(kb[0][:, :mg], b16[0:P, :mg])
        nc.scalar.dma_start(ka[0][:, : n // 2], a16[0:P, : n // 2])
        nc.sync.dma_start(kb[0][:, mg:], b16[0:P, mg:])
        nc.scalar.dma_start(ka[0][:, n // 2 :], a16[0:P, n // 2 :])
        for k in range(1, k16):
            nc.sync.dma_start(kb[k], b16[k * P : (k + 1) * P, :])
            nc.scalar.dma_start(ka[k], a16[k * P : (k + 1) * P, :])
        if fp8:
            nc.gpsimd.dma_start(t8b, ins["b8"][:])
            nc.gpsimd.dma_start(t8a, ins["a8"][:])

        # Unit order: g-major, nb within; sets of 8 units = 8 PSUM banks.
        # The final 8 units run as two 4-unit sets so the very last
        # drain (on the critical tail) covers fewer copies.
        units = [(g, nb) for g in range(m_groups) for nb in range(n_blocks)]
        sets = [units[i : i + 8] for i in range(0, len(units) - 8, 8)]
        sets += [units[-8:-4], units[-4:]]
        for si, su in enumerate(sets):
            # PSUM tiles allocated lazily (at first use) so each round-0
            # matmul waits only on its own bank's drain, not all eight.
            pss = [None] * len(su)
            for r in range(rounds):
                if fp8 and r == rounds - 1:
                    for u, (g, nb) in enumerate(su):
                        nc.tensor.matmul(
                            pss[u],
                            t8a[:, :, nb * P : (nb + 1) * P],
                            t8b[:, :, g * mg : (g + 1) * mg],
                            start=False,
                            stop=True,
                            perf_mode=mybir.MatmulPerfMode.DoubleRow,
                        )
                else:
                    for u, (g, nb) in enumerate(su):
                        if r == 0:
                            pss[u] = mpsum_pool.tile(
                                [P, mg], f32, tag="ps", name=f"ps{si}_{u}"
                            )
                        nc.tensor.matmul(
                            pss[u],
                            ka[r][:, nb * P : (nb + 1) * P],
                            kb[r][:, g * mg : (g + 1) * mg],
                            start=(r == 0),
                            stop=(r == rounds - 1),
                        )
            # Drain: copies alternate vector/scalar into merged staging
            # tiles; one output DMA per 4 units (few large DMAs keep slow
            # DMA-completion semaphores off the critical path).  The last
            # set instead streams per-unit DMAs so the final transfer is
            # small and starts as early as possible (shortens the tail).
            # c is laid out [P, n_blocks, m]: c[p, nb, col] = C[nb*P+p, col]
            for half, qh in ((0, nc.sync), (4, nc.scalar)):
                hu = su[half : half + 4]
                if not hu:
                    continue
                if len(su) == 4 and si == len(sets) - 1:
                    # Final 4-unit set: issue on scalar so the last two
                    # output DMAs ride different queues and overlap.
                    qh = nc.scalar
                ot = co_pool.tile([P, 4, mg], dt16, tag="ot")
                for j, (g, nb) in enumerate(hu):
                    if j % 2 == 0:
                        nc.vector.tensor_copy(ot[:, j, :], pss[half + j])
                    else:
                        nc.scalar.copy(ot[:, j, :], pss[half + j])
                gh, nbh = hu[0]
                qh.dma_start(
                    c[:, nbh : nbh + 4, gh * mg : (gh + 1) * mg], ot
                )


# --------------------------------------------------------------------------
# XBAR path (fp16x): host pre-cast fp16, on-chip DMA-transpose (fallback)
# --------------------------------------------------------------------------
def qkt_kernel_xbar(tc, ins, c, n, m, e, dt16, mg=512):
    nc = tc.nc
    f32 = mybir.dt.float32
    mg = min(mg, m)
    n_blocks = n // P
    e_chunks = e // P
    m_groups = m // mg
    a_hi, b_hi = ins
    srcs = {"ah": a_hi, "bh": b_hi}

    with (
        tc.tile_pool(name="tpt", bufs=1) as tp_pool,
        tc.tile_pool(name="mpsum", bufs=8, space="PSUM") as mpsum_pool,
        tc.tile_pool(name="co", bufs=4) as co_pool,
    ):
        tchunks = {tag: [None] * e_chunks for tag in srcs}
        for k in range(e_chunks):
            for tag, src in srcs.items():
                rows = n if tag[0] == "a" else m
                t = tp_pool.tile([P, rows], dt16, name=f"t_{tag}{k}")
                nc.sync.dma_start_transpose(t, src[:, k * P : (k + 1) * P])
                tchunks[tag][k] = t

        units = [(g, nb) for g in range(m_groups) for nb in range(n_blocks)]
        sets = [units[i0 : i0 + 8] for i0 in range(0, len(units), 8)]
        for si, chunk_units in enumerate(sets):
            pss = [
                mpsum_pool.tile([P, mg], f32, tag="ps", name=f"ps_{si}_{u}")
                for u in range(len(chunk_units))
            ]
            for k in range(e_chunks):
                for u, (g, nb) in enumerate(chunk_units):
                    nc.tensor.matmul(
                        pss[u],
                        tchunks["ah"][k][:, nb * P : (nb + 1) * P],
                        tchunks["bh"][k][:, g * mg : (g + 1) * mg],
                        start=(k == 0),
                        stop=(k == e_chunks - 1),
                    )
            for u, (g, nb) in enumerate(chunk_units):
                ot = co_pool.tile([P, mg], f32, tag="ot")
                nc.vector.tensor_copy(ot, pss[u])
                nc.scalar.dma_start(
                    c[nb * P : (nb + 1) * P, g * mg : (g + 1) * mg], ot
                )


# --------------------------------------------------------------------------
# Builders
# --------------------------------------------------------------------------
def build_qkt(n, m, e, mm_mode="fp16t", mg=512, fillers=None):
    f32 = mybir.dt.float32
    f16 = mybir.dt.float16
    f8 = mybir.dt.float8e4
    nc = bacc.Bacc(None, target_bir_lowering=False)
    with tile.TileContext(nc) as tc:
        with tc.tile_pool(name="dram", bufs=1, space="DRAM") as dram:
            if mm_mode in ("v2", "v2m8", "v2m12", "v2m16", "v3", "v3x",
                           "v3z"):
                dr4 = {"v2": 0, "v2m8": 2, "v2m12": 3, "v2m16": 4,
                       "v3": 0, "v3x": 0, "v3z": 0}[mm_mode]
                # v3: sets 0-1 (tiles 0-15) stay fp16 (they are DMA-feed
                # bound at kernel start — fp8 there saves no wall time);
                # tiles 16+ get 2 fp8 pairs, every 3rd gets 1, spending
                # the error budget where the stream is compute-bound.
                # Global fp8 fraction 80*256/65536 = 0.3125 -> rel err
                # ~1.78e-2 (deterministic), gate 2e-2.
                dr_plan = None
                if mm_mode == "v3":
                    dr_plan = [
                        0 if i < 16 else (1 if (i - 16) % 3 == 2 else 2)
                        for i in range(64)
                    ]
                elif mm_mode == "v3z":
                    # all 48 late tiles with 2 pairs: f=0.375, rel err
                    # 1.594e-2*sqrt(0.375/0.25) = 1.951e-2 (< 2e-2,
                    # deterministic)
                    dr_plan = [0 if i < 16 else 2 for i in range(64)]
                elif mm_mode == "v3x":
                    # 40 late tiles with 2 pairs, 8 with 1: f=0.34375,
                    # rel err 1.594e-2*sqrt(f/0.25) = 1.87e-2 (< 2e-2,
                    # deterministic)
                    dr_plan = [
                        0 if i < 16 else (1 if (i - 16) % 6 == 5 else 2)
                        for i in range(64)
                    ]
                c = dram.tile(
                    [P, n // P, m], f16, kind="ExternalOutput", name="out"
                )
                handles = {
                    "a3": dram.tile(
                        [P, e // P, n], f16, kind="ExternalInput", name="a3"
                    ),
                    "b3": dram.tile(
                        [P, e // P, m], f16, kind="ExternalInput", name="b3"
                    ),
                }
                if dr4 or dr_plan:
                    handles["a8"] = dram.tile(
                        [P, 2, n], f8, kind="ExternalInput", name="a8"
                    )
                    handles["b8"] = dram.tile(
                        [P, 2, m], f8, kind="ExternalInput", name="b8"
                    )
                if dr_plan and max(dr_plan) > 1:
                    handles["a82"] = dram.tile(
                        [P, 2, n], f8, kind="ExternalInput", name="a82"
                    )
                    handles["b82"] = dram.tile(
                        [P, 2, m], f8, kind="ExternalInput", name="b82"
                    )
                qkt_kernel_v2(
                    tc,
                    {k: h[:] for k, h in handles.items()},
                    c[:],
                    n,
                    m,
                    e,
                    f16,
                    mg=mg,
                    fillers=fillers if fillers is not None else 20,
                    dr4=dr4,
                    dr_plan=dr_plan,
                )
                in_names = {k: h.name for k, h in handles.items()}
            elif mm_mode in ("fp16t", "mix8"):
                fp8 = mm_mode == "mix8"
                e16 = e - (FP8_COLS if fp8 else 0)
                c = dram.tile(
                    [P, n // P, m], f16, kind="ExternalOutput", name="out"
                )
                handles = {
                    "a16": dram.tile([e16, n], f16, kind="ExternalInput", name="a16"),
                    "b16": dram.tile([e16, m], f16, kind="ExternalInput", name="b16"),
                }
                if fp8:
                    handles["a8"] = dram.tile(
                        [P, 2, n], f8, kind="ExternalInput", name="a8"
                    )
                    handles["b8"] = dram.tile(
                        [P, 2, m], f8, kind="ExternalInput", name="b8"
                    )
                if dr_plan and max(dr_plan) > 1:
                    handles["a82"] = dram.tile(
                        [P, 2, n], f8, kind="ExternalInput", name="a82"
                    )
                    handles["b82"] = dram.tile(
                        [P, 2, m], f8, kind="ExternalInput", name="b82"
                    )
                qkt_kernel_pret(
                    tc,
                    {k: h[:] for k, h in handles.items()},
                    c[:],
                    n,
                    m,
                    e,
                    f16,
                    fp8,
                    mg=mg,
                    fillers=fillers,
                )
                in_names = {k: h.name for k, h in handles.items()}
            elif mm_mode == "fp16x":
                c = dram.tile([n, m], f32, kind="ExternalOutput", name="out")
                a = dram.tile([n, e], f16, kind="ExternalInput", name="a_hi")
                b = dram.tile([m, e], f16, kind="ExternalInput", name="b_hi")
                qkt_kernel_xbar(tc, [a[:], b[:]], c[:], n, m, e, f16, mg=mg)
                in_names = [a.name, b.name]
            else:
                raise ValueError(f"unknown mode {mm_mode}")
    nc.compile()
    return nc, in_names, c.name


_CACHE = {}


def _get_built(n, m, e, mm_mode, mg=512, fillers=None):
    key = (n, m, e, mm_mode, mg, fillers)
    if key not in _CACHE:
        _CACHE[key] = build_qkt(n, m, e, mm_mode=mm_mode, mg=mg, fillers=fillers)
    return _CACHE[key]


def _dr_interleave(xT8):
    """[256, r] fp8 (transposed cols 0:256) -> [128, 2, r] DR layout."""
    return np.ascontiguousarray(np.stack([xT8[:P], xT8[P : 2 * P]], axis=1))


def _pair_chunks(xT):
    """[e16, r] -> [e16//256, 128, 2, r]: chunk pairs, partition-major."""
    e16, r = xT.shape
    return np.ascontiguousarray(
        xT.reshape(e16 // (2 * P), 2, P, r).transpose(0, 2, 1, 3)
    )


def prep_inputs(mat_0, mat_1, mm_mode, in_names):
    """Host-side per-core input prep for each mode."""
    import ml_dtypes

    bsz = mat_0.shape[0]
    f16 = np.float16
    if mm_mode == "fp16x":
        a16 = mat_0.astype(f16)
        b16 = mat_1.astype(f16)
        return [{in_names[0]: a16[i], in_names[1]: b16[i]} for i in range(bsz)]

    f8 = ml_dtypes.float8_e4m3
    maps = []
    for i in range(bsz):
        aT = mat_0[i].T  # [e, n]
        bT = mat_1[i].T  # [e, m]
        if mm_mode in ("v2", "v2m8", "v2m12", "v2m16", "v3", "v3x", "v3z"):
            # [e, r] -> [P, e//P, r]: x3[p, k, col] = xT[k*P + p, col]
            n_, m_ = aT.shape[1], bT.shape[1]
            a3 = np.ascontiguousarray(
                aT.astype(f16).reshape(-1, P, n_).transpose(1, 0, 2)
            )
            b3 = np.ascontiguousarray(
                bT.astype(f16).reshape(-1, P, m_).transpose(1, 0, 2)
            )
            mp = {in_names["a3"]: a3, in_names["b3"]: b3}
            if mm_mode in ("v2m8", "v2m12", "v2m16", "v3", "v3x", "v3z"):
                e_ = aT.shape[0]
                mp[in_names["a8"]] = _dr_interleave(
                    aT[e_ - 2 * P :].astype(f8)
                )
                mp[in_names["b8"]] = _dr_interleave(
                    bT[e_ - 2 * P :].astype(f8)
                )
            if mm_mode in ("v3", "v3x", "v3z"):
                mp[in_names["a82"]] = _dr_interleave(
                    aT[e_ - 4 * P : e_ - 2 * P].astype(f8)
                )
                mp[in_names["b82"]] = _dr_interleave(
                    bT[e_ - 4 * P : e_ - 2 * P].astype(f8)
                )
            maps.append(mp)
        elif mm_mode == "fp16t":
            maps.append(
                {
                    in_names["a16"]: np.ascontiguousarray(aT.astype(f16)),
                    in_names["b16"]: np.ascontiguousarray(bT.astype(f16)),
                }
            )
        else:  # mix8
            maps.append(
                {
                    in_names["a16"]: np.ascontiguousarray(aT[FP8_COLS:].astype(f16)),
                    in_names["b16"]: np.ascontiguousarray(bT[FP8_COLS:].astype(f16)),
                    in_names["a8"]: _dr_interleave(aT[:FP8_COLS].astype(f8)),
                    in_names["b8"]: _dr_interleave(bT[:FP8_COLS].astype(f8)),
                }
            )
    return maps


def run_qkt(mat_0, mat_1, mm_mode="mix8", mg=512, fillers=None, trace=False):
    """Run the sharded kernel on full inputs [b, n, e], [b, m, e]."""
    bsz, n, e = mat_0.shape
    _, m, _ = mat_1.shape
    nc, in_names, c_name = _get_built(n, m, e, mm_mode, mg, fillers)
    in_maps = prep_inputs(mat_0, mat_1, mm_mode, in_names)
    res = run_bass_kernel_spmd(nc, in_maps, core_ids=list(range(bsz)), trace=trace)

    def unshard(r):
        o = r[c_name]
        if mm_mode in ("fp16t", "mix8", "v2", "v2m8", "v2m12", "v2m16",
                       "v3", "v3x", "v3z"):
            # [P, n_blocks, m] -> [n, m]
            o = o.transpose(1, 0, 2).reshape(n, m)
        return o.astype(np.float32)

    out = np.stack([unshard(res.results[i]) for i in range(bsz)], axis=0)
    return out, res


DEFAULT_MODE = "v3x"


def kernel(mat_0, mat_1):
    out, _ = run_qkt(
        np.asarray(mat_0, dtype=np.float32),
        np.asarray(mat_1, dtype=np.float32),
        mm_mode=DEFAULT_MODE,
    )
    return out

